# revision 1
# baseline (speedup 1.0000x reference)
"""LocallyConnected2d Bass kernel for 8 TRN2 NeuronCores.

Shapes (hardcoded): x (16,32,64,64) f32, weight (4096,288,64) f32,
bias (4096,64) f32 -> out (16,64,64,64) f32.

Strategy: shard the L=4096 spatial locations across 8 cores (512
locations = 8 output rows each).  Per location the compute is
(16x288)@(288x64)+bias.  K=288 is split into 3 chunks of 96 rows by
kernel-row ki (chunk ki holds (kj,c_in) pairs, 96 rows; chunk 2 gets a
97th "ones" row that multiplies a bias row folded into the weight
layout, so bias costs zero extra traffic).  Patches are the stationary
matmul operand (M=16 batch columns -> LDWEIGHTS is ~13ns), the 38MB/core
weight stream is the moving operand.  4 output rows are computed
concurrently via PE column-tiling (tile_position=(0,32g)), accumulating
into one (128,128) PSUM tile; a single DVE copy per 2-location step
moves PSUM into a (128,4096) output tile that DMAs back with contiguous
256B runs.
"""
import sys

sys.path.insert(0, "/opt/trn_rl_repo")
import numpy as np

_NC = [None]


def _build_nc():
    from concourse import bacc, mybir, tile

    f32 = mybir.dt.float32
    nc = bacc.Bacc("TRN2", target_bir_lowering=False, debug=False, num_devices=8)
    xp = nc.declare_dram_parameter("xp", [10, 32, 1056], f32, isOutput=False)
    w0 = nc.declare_dram_parameter("w0", [96, 2, 32, 512], f32, isOutput=False)
    w1 = nc.declare_dram_parameter("w1", [96, 2, 32, 512], f32, isOutput=False)
    w2 = nc.declare_dram_parameter("w2", [97, 2, 32, 512], f32, isOutput=False)
    ones = nc.declare_dram_parameter("ones", [1, 10240], f32, isOutput=False)
    out_d = nc.declare_dram_parameter("out", [16, 64, 8, 64], f32, isOutput=True)

    with tile.TileContext(nc) as tc:
        with (
            tc.tile_pool(name="rp", bufs=1) as rp,
            tc.tile_pool(name="wp", bufs=4) as wp,
            tc.tile_pool(name="op", bufs=2) as op,
            tc.tile_pool(name="pp", bufs=4, space="PSUM") as pp,
        ):
            # Patch tiles: Rall[32*kj+c, 1024*r + 16*j + b] = x_pad[8m+r, c, j+kj, b]
            Rall = rp.tile([97, 10240], f32)
            for r in range(10):
                for kj in range(3):
                    nc.sync.dma_start(
                        out=Rall[32 * kj : 32 * kj + 32, 1024 * r : 1024 * (r + 1)],
                        in_=xp[r, :, 16 * kj : 16 * kj + 1024],
                    )
            nc.sync.dma_start(out=Rall[96:97, :], in_=ones[:])

            wgeom = (96, 96, 97)
            for ig in range(2):
                O = op.tile([128, 4096], f32)
                for jj in range(32):
                    w0t = wp.tile([96, 512], f32, tag="w0t")
                    w1t = wp.tile([96, 512], f32, tag="w1t")
                    w2t = wp.tile([97, 512], f32, tag="w2t")
                    wts = (w0t, w1t, w2t)
                    for wd, wt in zip((w0, w1, w2), wts):
                        nc.sync.dma_start(out=wt[:, :], in_=wd[:, ig, jj, :])
                    ps = pp.tile([128, 128], f32)
                    for j2 in range(2):
                        j = 2 * jj + j2
                        for c in range(3):
                            rows = wgeom[c]
                            wt = wts[c]
                            for g in range(4):
                                rl = 4 * ig + g + c
                                col = 1024 * rl + 16 * j
                                nc.tensor.matmul(
                                    ps[32 * g : 32 * g + 16, 64 * j2 : 64 * j2 + 64],
                                    Rall[0:rows, col : col + 16],
                                    wt[0:rows, (j2 * 4 + g) * 64 : (j2 * 4 + g) * 64 + 64],
                                    start=(c == 0),
                                    stop=(c == 2),
                                    tile_position=(0, 32 * g),
                                )
                    # PSUM (128, (j2,o)) -> O columns (o,j) at j = 2jj+j2
                    src = ps[:, :].rearrange("p (a b) -> p a b", a=2)
                    dst = O.rearrange("p (o j) -> p j o", j=64)[:, 2 * jj : 2 * jj + 2, :]
                    nc.vector.tensor_copy(out=dst, in_=src)
                for g in range(4):
                    il = 4 * ig + g
                    nc.sync.dma_start(
                        out=out_d[:, :, il, :],
                        in_=O[32 * g : 32 * g + 16, :].rearrange("p (o j) -> p o j", o=64),
                    )
    nc.compile()
    return nc


def _get_nc():
    if _NC[0] is None:
        _NC[0] = _build_nc()
    return _NC[0]


def _prep_maps(x, weight, bias):
    x = np.asarray(x, np.float32)
    weight = np.asarray(weight, np.float32)
    bias = np.asarray(bias, np.float32)
    xpad = np.pad(x, ((0, 0), (0, 0), (1, 1), (1, 1)))
    xpt = xpad.transpose(2, 1, 3, 0).reshape(66, 32, 1056)  # (H+2, C, (W+2)*B)
    w6 = weight.reshape(64, 64, 32, 3, 3, 64)  # (i, j, c_in, ki, kj, o)
    b3 = bias.reshape(64, 64, 64)  # (i, j, o)
    onesv = np.ones((1, 10240), np.float32)
    maps = []
    for m in range(8):
        xp_m = np.ascontiguousarray(xpt[8 * m : 8 * m + 10])
        w6m = w6[8 * m : 8 * m + 8]  # (8i, 64j, 32c, 3ki, 3kj, 64o)
        ws = []
        for ki in range(3):
            wc = w6m[:, :, :, ki, :, :]  # (8i, 64j, 32c, 3kj, 64o)
            wc = wc.reshape(2, 4, 32, 2, 32, 3, 64)  # (ig, g, jj, j2, c, kj, o)
            wc = wc.transpose(5, 4, 0, 2, 3, 1, 6)  # (kj, c, ig, jj, j2, g, o)
            ws.append(np.ascontiguousarray(wc.reshape(96, 2, 32, 512)))
        b3m = b3[8 * m : 8 * m + 8].reshape(2, 4, 32, 2, 64)  # (ig, g, jj, j2, o)
        brow = b3m.transpose(0, 2, 3, 1, 4).reshape(1, 2, 32, 512)  # (ig, jj, j2, g, o)
        w2b = np.ascontiguousarray(np.concatenate([ws[2], brow], axis=0))
        maps.append(
            {
                "xp": xp_m,
                "w0": ws[0],
                "w1": ws[1],
                "w2": w2b,
                "ones": onesv,
            }
        )
    return maps


def kernel(x, weight, bias):
    from concourse.bass_utils import run_bass_kernel_spmd

    nc = _get_nc()
    maps = _prep_maps(x, weight, bias)
    res = run_bass_kernel_spmd(nc, maps, core_ids=list(range(8)))
    outs = [res.results[m]["out"] for m in range(8)]
    return np.concatenate(outs, axis=2)
